# revision 32
# baseline (speedup 1.0000x reference)
"""Trainium2 Bass kernel for the BAHDANAU+ group-recommendation model (v4).

kernel(**inputs) takes the complete (unsharded) numpy inputs, distributes the
131072-query batch over 8 NeuronCores, runs the Bass kernel SPMD, and returns
the full [B, 1] float32 output.

Architecture (v4):
  Host-side TABLE transforms (all query-independent):
    group_tab[g] (256B rows, bf16): cols 0:96 = user_emb[members[g]] flat,
    96:99 = A_g = mem_flat @ attn_W[0:96] + attn_b, 99:123 = R_g =
    per-member mem_k @ pred_W1[32:64] ([3,8] k-major).  item_tab[i] (256B
    stride, 128B payload): cols 0:32 = item_emb||genres, 32:35 = B_i =
    it @ attn_W[96:128], 35:43 = Q_i = it @ pred_W1[64:96] + pred_b1.
    With these, at = A_g + B_i (attention logits as precomputed linear
    partials) and two of the three pred_W1 terms become DVE adds.

  Sharding: queries -> cores by GROUP range (62500 groups/core).  One
  dma_gather window with an idx-32768 base-slide covers the whole per-core
  group slice (signed-idx addressing in the SWDGE ucode), so positions need
  no group ordering.  Positions are sorted by ITEM id and segmented into 4
  contiguous item-quarter windows of 25000 rows (positive int16 indices).
  The ucode drops TRAILING negative indices of each gather, so host prep
  guarantees the last real position of every group-gather piece has a
  non-negative (rebased) index (swap or sentinel).

  Gathers: SWDGE dma_gather on FOUR queues (concurrent Q7 core pairs;
  measured ~3.8 ns/idx at 4 queues vs 11.5 serial).  512-idx pieces; group
  rows 256B, item rows 128B payload at 256B stride (direct InstDMAGatherAnt
  emission to relax the elem%256B assert).

  Compute (row-major): at = A_g+B_i; g = sum_k at_k mem_k; z = g*it;
  gw1b = at . R_g + Q_i (DVE); per tile on PE: transpose z, h8 = zT^T@W1a;
  h = relu(h8 + gw1b); y = sigmoid(sum h*W2 + b2).

  Output y is in permuted position order; the host scatters it back.
"""

import sys

sys.path.insert(0, "/opt/trn_rl_repo")

from contextlib import ExitStack

import numpy as np
import ml_dtypes

import concourse.bacc as bacc
import concourse.bass as bass
import concourse.tile as tile
from concourse import library_config, mybir
from concourse.ap_utils import ap_is_contiguous
from concourse.bass_utils import run_bass_kernel_spmd

N_CORES = 8
P = 128
EMB = 32
B = 131_072
NUM_USERS = 1_000_000
NUM_ITEMS = 100_000
NUM_GROUPS = 500_000
GPC = NUM_GROUPS // N_CORES        # groups per core (62500 < 65536)
IQ = NUM_ITEMS // 4                # item quarter-window (25000 < 32768)
SLIDE = 32_768                     # group idx base-slide
PIECE_T = 8                        # gather piece size in tiles (512 idx)
NQ = 4                             # SWDGE queues
CBLK = 8                           # compute-block tiles

GCOLS = 128                        # group row cols (bf16) = 256B
ICOLS = 64                         # item row payload cols = 128B
ISTEP = 128                        # item row stride cols = 256B

F32 = mybir.dt.float32
BF16 = mybir.dt.bfloat16
I16 = mybir.dt.int16
MULT = mybir.AluOpType.mult
ADD = mybir.AluOpType.add
AXX = mybir.AxisListType.X


def emit_gather(gp, out_ap, in_ap, idxs_ap, num_idxs, elem_size, queue_num,
                elem_step=None):
    """BassGpSimd.dma_gather (non-transpose, DRAM src) without the elem%256B
    assert; elem_step = row stride in elements (stride bytes must be %256)."""
    assert idxs_ap.dtype == mybir.dt.int16
    assert in_ap.dtype == out_ap.dtype
    assert in_ap.space == bass.MemorySpace.DRAM
    assert idxs_ap.space == bass.MemorySpace.SBUF
    assert out_ap.space == bass.MemorySpace.SBUF
    assert ap_is_contiguous(in_ap.ap[1:])
    assert ap_is_contiguous(out_ap.ap[1:])
    assert ap_is_contiguous(idxs_ap.ap[1:])
    if elem_step is None:
        elem_step = elem_size
    assert out_ap.ap[-1][1] == elem_size
    assert in_ap.ap[0][0] == elem_step
    assert out_ap.ap[0][1] * out_ap.ap[1][1] == ((num_idxs + P - 1) // P) * P
    stride_bytes = elem_step * mybir.dt.size(in_ap.dtype)
    assert stride_bytes % 256 == 0 and stride_bytes // 256 < 256
    _in_ap = gp.lower_ap_dma(in_ap, for_custom_bir_dma=True)
    inst = gp.add_instruction(
        mybir.InstDMAGatherAnt(
            name=gp.bass.get_next_instruction_name(),
            ins=[*_in_ap, gp.lower_ap(idxs_ap),
                 gp.lower_val_access(gp.to_reg(num_idxs))],
            outs=[gp.lower_ap(out_ap)],
            transpose=False,
            num_idxs=num_idxs,
            elem_size=elem_size,
            stride_bytes_256=stride_bytes // 256,
            gen_mode=0,
            single_packet=True,
            queue_num=queue_num,
            sbuf_tokens_per_rank=0,
            sbuf_free_dim_per_rank=0,
            sbuf_free_dim_pad_per_rank=0,
            sbuf_byte_offset=0,
        )
    )
    return inst.annotate(f"swdge_q={queue_num}")


def seg_pieces(ts):
    """Tile-ranges of gather pieces within one ts-tile segment."""
    out = []
    t = 0
    while t < ts:
        n = min(PIECE_T, ts - t)
        out.append((t, t + n))
        t += n
    return out


def build(ts, gathers_only=False):
    """Per-core program; ts = tiles per item-quarter segment (%4)."""
    nt = 4 * ts
    assert nt % CBLK == 0
    npos = nt * P
    nblk = nt // CBLK

    nc = bacc.Bacc(
        "TRN2",
        target_bir_lowering=False,
        debug=False,
        enable_asserts=False,
        num_swdge_queues=NQ,
    )

    gidx = nc.dram_tensor("gidx", [P, npos // 16], I16, kind="ExternalInput")
    iidx = nc.dram_tensor("iidx", [P, npos // 16], I16, kind="ExternalInput")
    gslice = nc.dram_tensor("gslice", [GPC, GCOLS], BF16, kind="ExternalInput")
    item_tab = nc.dram_tensor("item_tab", [NUM_ITEMS, ISTEP], BF16,
                              kind="ExternalInput")
    ident_d = nc.dram_tensor("ident", [P, P], BF16, kind="ExternalInput")
    w1a_d = nc.dram_tensor("w1a", [P, 4 * 8], BF16, kind="ExternalInput")
    w2_d = nc.dram_tensor("w2", [P, 8], F32, kind="ExternalInput")
    b2_d = nc.dram_tensor("b2", [P, 1], F32, kind="ExternalInput")
    y_out = nc.dram_tensor("y_out", [P, nt], F32, kind="ExternalOutput")

    with tile.TileContext(nc) as tc, ExitStack() as ctx:
        singles = ctx.enter_context(tc.tile_pool(name="singles", bufs=1))
        dve_p = ctx.enter_context(tc.tile_pool(name="dve", bufs=2))
        zt_p = ctx.enter_context(tc.tile_pool(name="zt", bufs=2))
        tp_ps = ctx.enter_context(
            tc.tile_pool(name="tp_ps", bufs=2, space=bass.MemorySpace.PSUM)
        )
        h_ps_p = ctx.enter_context(
            tc.tile_pool(name="h_ps", bufs=2, space=bass.MemorySpace.PSUM)
        )

        # --- constants (idx tiles first: gathers wait on them). Keep the
        # total input dma_start count <= 8: Tile's HWDGE sem pool has 8
        # sems, and recycling one onto a later DMA makes the first gather
        # transitively wait for it.
        gidx_s = singles.tile([P, npos // 16], I16)
        iidx_s = singles.tile([P, npos // 16], I16)
        s0 = slice(0, ts * 8)
        sr = slice(ts * 8, npos // 16)
        nc.sync.dma_start(out=gidx_s[:, s0], in_=gidx.ap()[:, s0])
        nc.sync.dma_start(out=iidx_s[:, s0], in_=iidx.ap()[:, s0])
        nc.sync.dma_start(out=gidx_s[:, sr], in_=gidx.ap()[:, sr])
        nc.sync.dma_start(out=iidx_s[:, sr], in_=iidx.ap()[:, sr])
        nc.gpsimd.load_library(library_config.mlp)
        ident = singles.tile([P, P], BF16)
        nc.sync.dma_start(out=ident[:], in_=ident_d.ap())
        zeros8 = singles.tile([P, 8], F32)
        nc.vector.memset(zeros8[:], 0)
        w1a_s = singles.tile([P, 4 * 8], BF16)
        nc.sync.dma_start(out=w1a_s[:], in_=w1a_d.ap())
        w2_s = singles.tile([P, 8], F32)
        nc.sync.dma_start(out=w2_s[:], in_=w2_d.ap())
        b2_s = singles.tile([P, 1], F32)
        nc.sync.dma_start(out=b2_s[:], in_=b2_d.ap())

        gdst = singles.tile([P, nt, GCOLS], BF16)
        idst = singles.tile([P, nt, ICOLS], BF16)
        ypre = singles.tile([P, nt], F32)

        # --- gathers: pieces over 4 queues -----------------------------
        # group piece p -> queue p%4, item piece p -> queue (p+2)%4, so both
        # tables' (different-sized) descriptors spread over all queues.
        g_base = gslice.ap()[SLIDE:, :]
        pn = 0
        for k in range(4):
            i_base = item_tab.ap()[k * IQ:, :]
            for (a, b) in seg_pieces(ts):
                t0, t1 = k * ts + a, k * ts + b
                n_idx = (t1 - t0) * P
                emit_gather(nc.gpsimd, gdst[:, t0:t1, :], g_base,
                            gidx_s[:, t0 * 8:t1 * 8], n_idx, GCOLS, pn % NQ)
                emit_gather(nc.gpsimd, idst[:, t0:t1, :], i_base,
                            iidx_s[:, t0 * 8:t1 * 8], n_idx, ICOLS,
                            (pn + 2) % NQ, elem_step=ISTEP)
                pn += 1

        # --- compute per block of CBLK tiles ---------------------------
        for blk in range(nblk):
            sl = slice(blk * CBLK, (blk + 1) * CBLK)
            gb = gdst[:, sl, :]
            ib = idst[:, sl, :]

            if gathers_only:
                nc.vector.tensor_reduce(out=ypre[:, sl], in_=gb[:, :, 0:8],
                                        axis=AXX, op=ADD)
                continue

            # at = A_g + B_i  [P, CBLK, 3]
            at = dve_p.tile([P, CBLK, 3], BF16, tag="at")
            nc.vector.tensor_tensor(out=at[:], in0=gb[:, :, 96:99],
                                    in1=ib[:, :, 32:35], op=ADD)
            atb = at[:].unsqueeze(3)

            # g = sum_k at_k * mem_k  [P, CBLK, 32]  (one fused 3x32 mul)
            wm = dve_p.tile([P, CBLK, 3, EMB], BF16, tag="wm")
            nc.vector.tensor_tensor(
                out=wm[:],
                in0=gb[:, :, 0:3 * EMB].rearrange("p c (k e) -> p c k e", k=3),
                in1=atb.to_broadcast([P, CBLK, 3, EMB]), op=MULT)
            g_t = dve_p.tile([P, CBLK, EMB], BF16, tag="g")
            nc.vector.tensor_tensor(out=g_t[:], in0=wm[:, :, 0, :],
                                    in1=wm[:, :, 1, :], op=ADD)
            nc.vector.tensor_tensor(out=g_t[:], in0=g_t[:],
                                    in1=wm[:, :, 2, :], op=ADD)

            # z = g * it  [P, CBLK, 32]
            z_t = dve_p.tile([P, CBLK, EMB], BF16, tag="z")
            nc.vector.tensor_tensor(out=z_t[:], in0=g_t[:],
                                    in1=ib[:, :, 0:EMB], op=MULT)

            # gw1b = at . R_g + Q_i  [P, CBLK, 8]  (one fused 3x8 mul)
            wr = dve_p.tile([P, CBLK, 3, 8], F32, tag="wr")
            nc.vector.tensor_tensor(
                out=wr[:],
                in0=gb[:, :, 99:123].rearrange("p c (k e) -> p c k e", k=3),
                in1=atb.to_broadcast([P, CBLK, 3, 8]), op=MULT)
            gw = dve_p.tile([P, CBLK, 8], F32, tag="gw")
            nc.vector.tensor_tensor(out=gw[:], in0=wr[:, :, 0, :],
                                    in1=wr[:, :, 1, :], op=ADD)
            nc.vector.tensor_tensor(out=gw[:], in0=gw[:],
                                    in1=wr[:, :, 2, :], op=ADD)
            nc.vector.tensor_tensor(out=gw[:], in0=gw[:], in1=ib[:, :, 35:43],
                                    op=ADD)

            # PE: one batched 4-tile transpose (z4^T [128,128]), then one
            # matmul vs block-diag W1a -> 4 tiles' h8 side by side.
            # PSUM layout [P, nb4, 4, 8] == [P, CBLK, 8] bit-for-bit.
            h_ps = h_ps_p.tile([P, CBLK, 8], F32, tag="h")
            for g0 in range(0, CBLK, 4):
                pst = tp_ps.tile([P, P], BF16, tag="tp")
                nc.tensor.matmul(
                    pst[:], lhsT=z_t[:, g0:g0 + 4, :], rhs=ident[:],
                    is_transpose=True, start=True, stop=True,
                    skip_group_check=True,
                )
                zt_sb = zt_p.tile([P, P], BF16, tag="zt")
                nc.scalar.copy(out=zt_sb[:], in_=pst[:])
                nc.tensor.matmul(
                    h_ps[:, g0:g0 + 4, :], lhsT=zt_sb[:], rhs=w1a_s[:],
                    start=True, stop=True, skip_group_check=True,
                )

            # h = relu(h8 + gw)  [P, CBLK, 8]
            h_sb = dve_p.tile([P, CBLK, 8], F32, tag="h_sb")
            nc.vector.tensor_tensor(out=h_sb[:], in0=h_ps[:], in1=gw[:], op=ADD)
            nc.vector.tensor_tensor(
                out=h_sb[:], in0=h_sb[:],
                in1=zeros8[:].unsqueeze(1).to_broadcast([P, CBLK, 8]),
                op=mybir.AluOpType.max)

            # y = sigmoid(sum h * w2 + b2), store per block
            hw = dve_p.tile([P, CBLK, 8], F32, tag="hw")
            nc.vector.tensor_tensor(
                out=hw[:], in0=h_sb[:],
                in1=w2_s[:].unsqueeze(1).to_broadcast([P, CBLK, 8]), op=MULT)
            nc.vector.tensor_reduce(out=ypre[:, sl], in_=hw[:], axis=AXX,
                                    op=ADD)
            ysig = dve_p.tile([P, CBLK], F32, tag="ysig")
            nc.scalar.activation(
                out=ysig[:], in_=ypre[:, sl],
                func=mybir.ActivationFunctionType.Sigmoid,
                bias=b2_s[:, 0:1], scale=1.0,
            )
            nc.sync.dma_start(out=y_out.ap()[:, sl], in_=ysig[:])

    nc.compile()
    return nc


def wrap_idx(vals):
    """[npos] -> [128, npos//16] int16: position j -> partition j%16
    (replicated across the 8 16-partition groups), column j//16."""
    npos = len(vals)
    block = vals.reshape(npos // 16, 16).T
    return np.ascontiguousarray(np.tile(block, (8, 1)))


def prep_host_inputs(inputs, n_cores=N_CORES):
    grp = np.asarray(inputs["group_inputs"]).astype(np.int64).reshape(-1)
    itm = np.asarray(inputs["item_inputs"]).astype(np.int64).reshape(-1)
    nq = grp.shape[0]

    user_emb = np.asarray(inputs["user_emb"], np.float32)
    members = np.asarray(inputs["members"]).astype(np.int64)
    attn_W = np.asarray(inputs["attn_W"], np.float32)
    attn_b = np.asarray(inputs["attn_b"], np.float32)
    w1 = np.asarray(inputs["pred_W1"], np.float32)
    b1 = np.asarray(inputs["pred_b1"], np.float32)
    w2 = np.asarray(inputs["pred_W2"], np.float32)
    b2 = np.asarray(inputs["pred_b2"], np.float32)

    # --- group table: mem | A_g | R_g ---------------------------------
    mem_flat = user_emb[members.reshape(-1)].reshape(NUM_GROUPS, 3 * EMB)
    group_tab = np.zeros((NUM_GROUPS, GCOLS), ml_dtypes.bfloat16)
    group_tab[:, :3 * EMB] = mem_flat.astype(ml_dtypes.bfloat16)
    a_g = mem_flat @ attn_W[:3 * EMB] + attn_b[None, :]        # [G, 3]
    group_tab[:, 96:99] = a_g.astype(ml_dtypes.bfloat16)
    w1b = w1[EMB:2 * EMB]                                      # [32, 8]
    r_g = np.einsum("gkc,cj->gkj",
                    mem_flat.reshape(NUM_GROUPS, 3, EMB), w1b)  # [G, 3, 8]
    group_tab[:, 99:123] = r_g.reshape(NUM_GROUPS, 24).astype(ml_dtypes.bfloat16)

    # --- item table: it | B_i | Q_i -----------------------------------
    it_rows = np.concatenate(
        [np.asarray(inputs["item_emb"], np.float32),
         np.asarray(inputs["genres"], np.float32)], axis=1)     # [I, 32]
    item_tab = np.zeros((NUM_ITEMS, ISTEP), ml_dtypes.bfloat16)
    item_tab[:, :EMB] = it_rows.astype(ml_dtypes.bfloat16)
    b_i = it_rows @ attn_W[3 * EMB:]                            # [I, 3]
    item_tab[:, 32:35] = b_i.astype(ml_dtypes.bfloat16)
    q_i = it_rows @ w1[2 * EMB:] + b1[None, :]                  # [I, 8]
    item_tab[:, 35:43] = q_i.astype(ml_dtypes.bfloat16)

    # --- assign queries to cores by group range; item-sorted segments --
    core_of = grp // GPC
    per_core = []
    max_seg = 1
    for c in range(n_cores):
        qc = np.nonzero(core_of == c)[0]
        qc = qc[np.argsort(itm[qc], kind="stable")]
        bounds = np.searchsorted(itm[qc], [0, IQ, 2 * IQ, 3 * IQ, NUM_ITEMS])
        segs = [qc[bounds[k]:bounds[k + 1]] for k in range(4)]
        per_core.append(segs)
        max_seg = max(max_seg, max(len(s) for s in segs))
    ts = -(-max_seg // P)
    ts += ts % 2                       # even -> nt % CBLK == 0
    npos = 4 * ts * P

    in_extra = []
    perms = []
    for c in range(n_cores):
        # Pads use index 0 (a valid row), NEVER negative: the ucode
        # self-trims trailing negative indices, which desyncs its
        # descriptor count from the decode-side ring reservation and
        # corrupts the SWDGE ring once it wraps (device fault).
        gl = np.zeros(npos, np.int16)
        il = np.zeros(npos, np.int16)
        pm = np.full(npos, -1, np.int64)
        for k, qs in enumerate(per_core[c]):
            o = k * ts * P
            n = len(qs)
            gl[o:o + n] = (grp[qs] - c * GPC - SLIDE).astype(np.int16)
            il[o:o + n] = (itm[qs] - k * IQ).astype(np.int16)
            pm[o:o + n] = qs
            # last position of each piece must have gidx >= 0 (no trim)
            for (a, b) in seg_pieces(ts):
                last = o + b * P - 1
                if gl[last] >= 0:
                    continue
                p0 = o + a * P
                cand = np.nonzero(gl[p0:last] >= 0)[0]
                assert len(cand), "all-negative gather piece"
                j = p0 + cand[-1]
                for arr in (gl, il, pm):
                    arr[j], arr[last] = arr[last], arr[j]
        in_extra.append({"gidx": wrap_idx(gl), "iidx": wrap_idx(il)})
        perms.append(pm)

    gslices = [np.ascontiguousarray(group_tab[c * GPC:(c + 1) * GPC])
               for c in range(n_cores)]
    w1abd = np.zeros((P, 32), np.float32)
    for k in range(4):
        w1abd[32 * k:32 * k + 32, 8 * k:8 * k + 8] = w1[:EMB]
    weights = {
        "item_tab": item_tab,
        "ident": np.ascontiguousarray(np.eye(P, dtype=ml_dtypes.bfloat16)),
        "w1a": np.ascontiguousarray(w1abd.astype(ml_dtypes.bfloat16)),
        "w2": np.ascontiguousarray(np.tile(w2[:, 0][None, :], (P, 1))),
        "b2": np.ascontiguousarray(np.tile(b2.reshape(1, 1), (P, 1))),
    }
    return gslices, weights, in_extra, perms, ts, nq


def make_in_maps(gslices, weights, in_extra):
    return [{"gslice": gslices[c], **weights, **ex}
            for c, ex in enumerate(in_extra)]


_NC_CACHE = {}


def kernel(**inputs) -> np.ndarray:
    gslices, weights, in_extra, perms, ts, nq = prep_host_inputs(inputs)
    if ts not in _NC_CACHE:
        _NC_CACHE[ts] = build(ts)
    nc = _NC_CACHE[ts]
    in_maps = make_in_maps(gslices, weights, in_extra)
    res = run_bass_kernel_spmd(nc, in_maps, core_ids=list(range(N_CORES)))
    y = np.zeros(nq, np.float32)
    for c in range(N_CORES):
        yc = res.results[c]["y_out"]   # [128, nt]; position j -> [j%128, j//128]
        flat = np.ascontiguousarray(yc.T).reshape(-1)
        pm = perms[c]
        valid = pm >= 0
        y[pm[valid]] = flat[valid]
    return y.reshape(-1, 1).astype(np.float32)
